# revision 43
# baseline (speedup 1.0000x reference)
"""AttentionRouter Trainium2 kernel.

Computes, for packed tokens x [T=32768, H=8, D=128] with B=8 ragged segments
(cu_seq_len [9]), the per-segment mean-pooled features -> tiny MLP router ->
binary mask z [B, H, 1].

Strategy (8 NeuronCores, data-parallel over tokens):
  - Each core owns 4096 tokens (16 MiB of x), streamed f32 as 16 x 1 MiB
    chunks alternating the two HWDGE rings (one ring tops out at ~170 GB/s;
    two saturate the ~358 GB/s per-NC HBM limit), consumed by the PE as
    float32r (same 4-byte data, single-pass matmul at moving dim >= 256).
    At most ~8 HWDGE DMAs per engine stay outstanding so issue never stalls
    on completion-semaphore lane reuse.
  - Segment membership masks are precomputed on host (O(T*B) metadata, like
    an attention mask) with 1/(H*max(count,1)) folded into the mask values,
    so the phase-1 mask-matmuls accumulate mean contributions directly.
  - Both feature halves accumulate into one PSUM bank so half the head
    reduction is free; remaining head-sum is 3 DVE ops.
  - All MLP weights/biases + the selector ride in ONE host-prepped bf16
    blob as the LAST entry on the sync HWDGE ring: ring-FIFO lands it right
    after the stream, ahead of the MLP's first use, and keeps it off the
    SWDGE queue (whose completion-semaphore lane stalled the x issues and
    whose DRAIN gates the collective trigger).
  - A tiny bf16 AllGather combines the pre-scaled partials. A same-shape
    warm-up gather is issued at the top: the CC stream runs a ~50us
    global-comm BARRIER (rendezvous + setup) either way, but without the
    warm-up the first real op's processing measured 5-27us instead of
    ~6.5us. The gathered [64,128] returns as two halves so the first
    "selector" matmul overlaps the second half's DMA receipt; the selector
    (a 0/1 diagonal stack) sums the 8 cores' partials AND transposes the
    pooled mean to [feat, batch] on the PE.
  - Every core then (redundantly) runs the 5-layer MLP in bf16 with biases
    applied on the ACT engine (out = silu(psum + bias)); the final layer
    uses the activations as the stationary operand to emit logits [8, 2]
    directly (no transpose), and z = (logit1 > logit0) is stored [8, 1].
    The host takes core 0's output and broadcasts to [B, H, 1].
"""

import sys

if "/opt/trn_rl_repo" not in sys.path:
    sys.path.insert(0, "/opt/trn_rl_repo")

import numpy as np

import concourse.bacc as bacc
import concourse.tile as tile
from concourse import mybir
from concourse.bass_utils import run_bass_kernel_spmd

N_CORES = 8
T, B, H, D = 32768, 8, 8, 128
E = H * D                      # 1024 features per token (heads folded in)
TOK = T // N_CORES             # 4096 tokens per core
NPART = 128
TPB = TOK // NPART             # 32 token-blocks (matmul contraction tiles)
NCHUNK = 16                    # x DMA chunks per core (1 MiB each)
BPC = TPB // NCHUNK            # token-blocks per DMA chunk

F32 = mybir.dt.float32
F32R = mybir.dt.float32r
BF16 = mybir.dt.bfloat16

# ---- weight blob layout (bf16 cols per partition; see make_in_maps) ----
_WSPEC = [            # name, kch, M  (rearranged (k p) m -> p (k m))
    ("w1", 1, 8 * D),
    ("w2", 8, 2 * D),
    ("w3", 2, 4 * D),
    ("w4", 4, D),
]
_BSPEC = [("b1", 8), ("b2", 2), ("b3", 4), ("b4", 1)]  # [128, mch] chunks
_WCOLS = sum(k * m for _, k, m in _WSPEC)              # 4608
_SELC = _WCOLS                                         # sel [64, 8] at cols 4608..4616
_W5C = _SELC + 8                                       # w5 [128, 2]
_BC = {}
_c = _W5C + 2
for _n, _m in _BSPEC:
    _BC[_n] = _c
    _c += _m
_B5C = _c                                              # b5 [1, 2]
_BLOBC = _B5C + 2                                      # 4633 cols total


def _mlp_layer(nc, pp, sp, a_in, w_sb, b_col, K, M, act):
    """out[M, 8] = act(W.T @ a_in + b), activations transposed [feat, batch].
    a_in: [128, kch*8] bf16, chunk k at cols [k*8,(k+1)*8). w_sb: [128, kch, M]
    bf16. b_col: [128, mch] bf16 (feature-major bias chunks). Returns bf16
    [128, mch*8]."""
    kch = K // 128
    mch = (M + 127) // 128
    a_out = sp.tile([128, mch * 8], BF16, tag="act")
    for m in range(mch):
        mm = min(128, M - m * 128)
        ps = pp.tile([128, 8], F32, tag="mlp_ps")
        for k in range(kch):
            nc.tensor.matmul(
                ps[0:mm, :],
                w_sb[:, k, m * 128 : m * 128 + mm],
                a_in[:, k * 8 : (k + 1) * 8],
                start=(k == 0),
                stop=(k == kch - 1),
            )
        fn = (
            mybir.ActivationFunctionType.Silu
            if act
            else mybir.ActivationFunctionType.Identity
        )
        nc.scalar.activation(
            a_out[0:mm, m * 8 : (m + 1) * 8], ps[0:mm, :], fn,
            bias=b_col[0:mm, m : m + 1],
        )
    return a_out


def _build_kernel_body(nc, tc, d):
    """d: dict of DRAM tensor handles."""
    with (
        tc.tile_pool(name="xp", bufs=NCHUNK) as xp,
        tc.tile_pool(name="wp", bufs=1) as wp,
        tc.tile_pool(name="sp", bufs=1) as sp,
        tc.tile_pool(name="spa", bufs=2) as spa,
        tc.tile_pool(name="pp", bufs=1, space="PSUM") as pp,
        tc.tile_pool(name="ppm", bufs=4, space="PSUM") as ppm,
        tc.tile_pool(name="dp", bufs=1, space="DRAM") as dp,
    ):
        # ---- warm-up collective FIRST. cc_ops shows the CC stream runs a
        # ~50us global-comm BARRIER (rendezvous + setup) regardless, ending
        # ~71us; the warm-up gather then keeps the stream pipeline hot —
        # without it the real gather's first-op processing measured 5-27us
        # instead of ~6.5us. ----
        wusrc = sp.tile([8, D], BF16)
        nc.vector.memset(wusrc[:], 0.0)
        wuin = dp.tile([8, D], BF16)
        wuout = dp.tile([N_CORES * 8, D], BF16)
        nc.gpsimd.dma_start(wuin[:], wusrc[:])
        nc.gpsimd.collective_compute(
            "AllGather",
            mybir.AluOpType.bypass,
            replica_groups=[list(range(N_CORES))],
            ins=[wuin.opt()],
            outs=[wuout.opt()],
        )

        # ---- x chunk DMAs issued next, alternating the two HWDGE rings
        # (SP + ACT); chunk c -> ring c%2 so completions interleave in
        # consumption order and the phase-1 matmuls pipeline chunk-by-chunk
        # behind the stream. The mask rides the sync ring ahead of its
        # first chunk. ----
        mask = sp.tile([128, B, TPB], F32R)
        nc.sync.dma_start(mask[:], d["mask"].ap().rearrange("p (s n) -> p s n", s=B))
        xv = d["x"].ap().rearrange("(p n) e -> p n e", p=128)
        xts = []
        for c in range(NCHUNK):
            xf = xp.tile([128, BPC, E], F32R, tag="xf", name=f"xf{c}")
            eng = nc.sync if c % 2 == 0 else nc.scalar
            eng.dma_start(xf[:], xv[:, c * BPC : (c + 1) * BPC, :])
            xts.append((xf, c * BPC, BPC))

        # ---- the single bf16 blob: all weights + biases + selector, as
        # the LAST entry on the sync ring: ring-FIFO lands it right after
        # the stream (~52us), ahead of the MLP's first use (~60us). Keeping
        # it off the gpsimd SWDGE queue matters twice over: the warm
        # collective's trigger DRAINs that queue, and a SWDGE blob's
        # completion-semaphore lane stalled the x-chunk issues (observed
        # +9us on the first phase-1 matmul). ----
        blob = wp.tile([128, _BLOBC], BF16)
        nc.sync.dma_start(blob[:], d["blob"].ap())

        ones_row = sp.tile([1, 8], BF16)
        nc.vector.memset(ones_row[:], 1.0)
        # dummy Silu early in the ACT queue: forces its ACT_TABLE_LOAD to
        # run at kernel start instead of just-in-time on the MLP critical
        # path (observed +1.3us otherwise)
        act_warm = sp.tile([1, 8], BF16)
        nc.scalar.activation(
            act_warm[:], ones_row[:], mybir.ActivationFunctionType.Silu
        )
        # dummy matmul with no x dependency: the ncfw global-comm BARRIER
        # (49.6us on the CC stream, observed in cc_ops) is triggered from
        # the Tensor queue and otherwise sits behind the first phase-1
        # matmul's chunk wait (~21.5us); this pulls it to ~7us so the CC
        # stream is free before the partials arrive
        pe_warm = ppm.tile([8, 8], F32, tag="mlp_ps")
        nc.tensor.matmul(pe_warm[:], ones_row[:], ones_row[:], start=True, stop=True)

        # ---- phase 1: masked (pre-scaled) segment means over this core's
        # tokens. x viewed [128, TPB, E]: partition p, block n holds token
        # p*TPB + n. Both feature halves accumulate into ONE psum bank:
        # psum[b, h'*128+d] = sum over heads h' and h'+4 — half the head
        # reduction happens for free in the PE accumulator. ----
        ps0 = pp.tile([B, 512], F32)
        for xf, n0, nb in xts:
            for k in range(nb):
                n = n0 + k
                first, last = (n == 0), (n == TPB - 1)
                lhsT = mask[:, :, n]
                nc.tensor.matmul(ps0[:], lhsT, xf[:, k, 0:512], start=first, stop=False)
                nc.tensor.matmul(ps0[:], lhsT, xf[:, k, 512:E], start=False, stop=last)

        w_sbs = {}
        c0 = 0
        for name, kch, M in _WSPEC:
            w_sbs[name] = blob[:, c0 : c0 + kch * M].rearrange(
                "p (k m) -> p k m", k=kch
            )
            c0 += kch * M
        sel_sb = blob[0 : N_CORES * B, _SELC : _SELC + 8]
        w5_sb = blob[:, _W5C : _W5C + 2]
        b_sbs = {n: blob[:, _BC[n] : _BC[n] + m] for n, m in _BSPEC}
        b5_sb = blob[0:1, _B5C : _B5C + 2]

        # ---- finish head-sum: [B, 512] -> [B, 128] (already mean-scaled;
        # tensor_tensor may read at most one input from PSUM, so copy first)
        s512 = sp.tile([B, 512], F32)
        nc.vector.tensor_copy(s512[:], ps0[:])
        s256 = sp.tile([B, 256], F32)
        nc.vector.tensor_tensor(
            s256[:], s512[:, 0:256], s512[:, 256:512], op=mybir.AluOpType.add
        )
        pre = sp.tile([B, D], BF16)
        nc.vector.tensor_tensor(
            pre[:], s256[:, 0:128], s256[:, 128:256], op=mybir.AluOpType.add
        )

        # ---- AllGather partial means across the 8 cores (bf16 payload;
        # cheaper ncfw path than AllReduce at this size) ----
        arin = dp.tile([B, D], BF16)
        arout = dp.tile([N_CORES * B, D], BF16, addr_space="Shared")
        nc.scalar.dma_start(arin[:], pre[:])
        nc.gpsimd.collective_compute(
            "AllGather",
            mybir.AluOpType.bypass,
            replica_groups=[list(range(N_CORES))],
            ins=[arin.opt()],
            outs=[arout.opt()],
        )
        # read the gathered partials back in two halves so the first
        # selector matmul overlaps the second half's DMA receipt
        post = sp.tile([N_CORES * B, D], BF16)
        nc.scalar.dma_start(post[0:32, :], arout[0:32, :])
        nc.scalar.dma_start(post[32:64, :], arout[32:64, :])
        # selector matmul: sums the 8 cores' partials AND transposes:
        # a0[feat, seg] = sum_c post[(c, seg), feat]
        ps_sel = ppm.tile([D, B], F32, tag="mlp_ps")
        nc.tensor.matmul(ps_sel[:], post[0:32, :], sel_sb[0:32, :], start=True, stop=False)
        nc.tensor.matmul(ps_sel[:], post[32:64, :], sel_sb[32:64, :], start=False, stop=True)
        a0 = spa.tile([D, B], BF16, tag="act")
        nc.vector.tensor_copy(a0[:], ps_sel[:])

        # ---- MLP (activations kept transposed: [feature, batch]) ----
        a1 = _mlp_layer(nc, ppm, spa, a0, w_sbs["w1"], b_sbs["b1"], D, 8 * D, True)
        a2 = _mlp_layer(nc, ppm, spa, a1, w_sbs["w2"], b_sbs["b2"], 8 * D, 2 * D, False)
        a3 = _mlp_layer(nc, ppm, spa, a2, w_sbs["w3"], b_sbs["b3"], 2 * D, 4 * D, True)
        a4 = _mlp_layer(nc, ppm, spa, a3, w_sbs["w4"], b_sbs["b4"], 4 * D, D, True)

        # ---- final layer: logits [8, 2] directly (a4 as stationary) ----
        ps5 = ppm.tile([B, 2], F32, tag="mlp_ps")
        nc.tensor.matmul(ps5[:], a4[:], w5_sb, start=True, stop=False)
        nc.tensor.matmul(ps5[:], ones_row[:], b5_sb, start=False, stop=True)
        lg = sp.tile([B, 2], F32)
        nc.vector.tensor_copy(lg[:], ps5[:])
        z = sp.tile([B, 1], F32)
        nc.vector.tensor_tensor(z[:], lg[:, 1:2], lg[:, 0:1], op=mybir.AluOpType.is_gt)
        nc.sync.dma_start(d["out"].ap(), z[:])


def build():
    nc = bacc.Bacc("TRN2", target_bir_lowering=False, debug=False, num_devices=N_CORES)
    d = {}
    d["x"] = nc.dram_tensor("x", [TOK, E], F32R, kind="ExternalInput")
    d["mask"] = nc.dram_tensor("mask", [NPART, B * TPB], F32R, kind="ExternalInput")
    d["blob"] = nc.dram_tensor("blob", [NPART, _BLOBC], BF16, kind="ExternalInput")
    d["out"] = nc.dram_tensor("out", [B, 1], F32, kind="ExternalOutput")
    with tile.TileContext(nc) as tc:
        _build_kernel_body(nc, tc, d)
    nc.compile()
    return nc


def make_in_maps(x, cu_seq_len, w1, b1, w2, b2, w3, b3, w4, b4, w5, b5):
    import ml_dtypes

    bf16 = np.dtype(ml_dtypes.bfloat16)
    x = np.ascontiguousarray(np.asarray(x, dtype=np.float32)).reshape(T, E)
    cu = np.asarray(cu_seq_len, dtype=np.int64)
    # per-token segment id and mean scale 1/(H * max(count, 1))
    seg_id = np.searchsorted(cu, np.arange(T), side="right") - 1   # [T]
    cnt = np.maximum(np.diff(cu).astype(np.float64), 1.0)          # [B]
    scale = (1.0 / (H * cnt)).astype(np.float32)                   # [B]
    onehot = (seg_id[:, None] == np.arange(B)[None, :])            # [T, B]
    mask_full = onehot * scale[None, :]                            # [T, B] f32

    # ---- bf16 blob: weights (k p) m -> p (k m), sel, w5, biases, b5 ----
    blob = np.zeros((NPART, _BLOBC), dtype=bf16)
    c0 = 0
    for w, (name, kch, M) in zip((w1, w2, w3, w4), _WSPEC):
        wa = np.asarray(w, np.float32).reshape(kch, NPART, M).transpose(1, 0, 2)
        blob[:, c0 : c0 + kch * M] = wa.reshape(NPART, kch * M).astype(bf16)
        c0 += kch * M
    blob[0 : N_CORES * B, _SELC : _SELC + 8] = np.tile(
        np.eye(B, dtype=np.float32), (N_CORES, 1)
    ).astype(bf16)
    blob[:, _W5C : _W5C + 2] = np.asarray(w5, np.float32).astype(bf16)
    for b, (name, mch) in zip((b1, b2, b3, b4), _BSPEC):
        ba = np.asarray(b, np.float32).reshape(mch, D).T               # [128, mch]
        blob[:, _BC[name] : _BC[name] + mch] = ba.astype(bf16)
    blob[0, _B5C : _B5C + 2] = np.asarray(b5, np.float32).astype(bf16)

    in_maps = []
    for c in range(N_CORES):
        # token (p, n) of this core = c*TOK + p*TPB + n; mask tile layout
        # [128, (s, n)] -> [p, s, n]
        mc = mask_full[c * TOK : (c + 1) * TOK].reshape(NPART, TPB, B)
        mc = np.ascontiguousarray(mc.transpose(0, 2, 1).reshape(NPART, B * TPB))
        in_maps.append({"x": x[c * TOK : (c + 1) * TOK], "mask": mc, "blob": blob})
    return in_maps


_NC_CACHE = {}


def _get_nc():
    if "nc" not in _NC_CACHE:
        _NC_CACHE["nc"] = build()
    return _NC_CACHE["nc"]


def kernel(**inputs):
    nc = _get_nc()
    in_maps = make_in_maps(**inputs)
    res = run_bass_kernel_spmd(nc, in_maps, core_ids=list(range(N_CORES)))
    z = np.asarray(res.results[0]["out"], dtype=np.float32).reshape(B, 1, 1)
    return np.ascontiguousarray(np.broadcast_to(z, (B, H, 1)))
